# revision 25
# baseline (speedup 1.0000x reference)
"""Trainium2 Bass kernel for nn_DevNet_63093069578584 (GAT row-op readout).

The reference computes two full GATConv layers (forward graph and reversed
graph) over N=100k nodes / E=1.6M edges but only reads row `op` of each
result, plus feat[op] and a 64-row feature sum.  Row `op` of a GAT depends
only on the edges incident to node `op` (expected ~16 of 1.6M), so the real
work is scanning the src/dst index arrays (2 x 6.4MB) for matches.

Distribution: edges are split evenly over 8 NeuronCores.  Each core scans
its chunk, extracts the matched neighbor ids (DVE top-8 per partition row,
then gpsimd sparse_gather compaction), the 8 cores AllGather their
candidate lists (32 floats each), and every core redundantly finishes the
tiny GAT math (indirect-DMA gather of <=128 feature rows + a few 128x128
matmuls) and writes the full [384] output.
"""

import os
import sys

import numpy as np

for _p in ("/opt/trn_rl_repo",):
    if _p not in sys.path:
        sys.path.insert(0, _p)

import concourse.bass as bass
import concourse.mybir as mybir
import concourse.tile as tile
from concourse import bacc
from concourse.bass_utils import run_bass_kernel_spmd
from concourse.masks import make_identity

# Problem constants (hardcoded per harness contract).
N = 100000
E = 1600000
IN = 128
H = 2
D = 64
NEG_SLOPE = 0.2
NCORES = 8
P = 128
EPC = E // NCORES          # edges per core = 200000
COLS = 1568                # free-dim columns; P*COLS = 200704 >= EPC
PADC = P * COLS
CAP = 16                   # candidate slots contributed per core per direction

F32 = mybir.dt.float32
I32 = mybir.dt.int32

AluOp = mybir.AluOpType
ActFn = mybir.ActivationFunctionType


def build_body(nc, tc, outs, ins):
    """Emit the kernel into TileContext `tc`.  `outs`/`ins` are dicts of DRAM APs."""
    out = outs["out"]

    with (
        tc.tile_pool(name="big", bufs=1) as big,
        tc.tile_pool(name="small", bufs=1) as small,
        tc.tile_pool(name="pp", bufs=1, space="PSUM") as pp,
        tc.tile_pool(name="dram", bufs=1, space="DRAM") as dram,
    ):
        # ---- shared small tiles -------------------------------------------------
        op_sb = small.tile([P, 1], F32, tag="op")
        nc.sync.dma_start(op_sb[:], ins["op_t"][:])

        ident = big.tile([P, P], F32, tag="ident")
        make_identity(nc, ident[:])

        ones = small.tile([P, 1], F32, tag="ones")
        nc.gpsimd.memset(ones[:], 1.0)

        # ---- phase A: scan edge chunks, per direction ---------------------------
        src_sb = big.tile([P, COLS], I32, tag="src")
        dst_sb = big.tile([P, COLS], I32, tag="dst")
        nc.sync.dma_start(src_sb[:], ins["src_t"][:])
        nc.sync.dma_start(dst_sb[:], ins["dst_t"][:])

        cc_in = dram.tile([2 * CAP], F32, tag="cc_in")
        cc_out = dram.tile([NCORES * 2 * CAP], F32, tag="cc_out")

        # slot index row for masking sparse_gather's garbage tail
        iota_row = small.tile([1, CAP], I32, tag="iota_row")
        nc.gpsimd.iota(iota_row[:], pattern=[[1, CAP]], base=0, channel_multiplier=0)

        masked_tiles = {}
        for di, d in enumerate(("f", "b")):
            cmp_sb = dst_sb if d == "f" else src_sb
            val_sb = src_sb if d == "f" else dst_sb

            eq = big.tile([P, COLS], I32, tag=f"eq_{d}")
            nc.vector.tensor_scalar(
                out=eq[:], in0=cmp_sb[:], scalar1=op_sb[:, :1], scalar2=None,
                op0=AluOp.is_equal,
            )

            cand = big.tile([P, COLS], I32, tag=f"cand_{d}")
            nc.gpsimd.memset(cand[:], -1)
            nc.vector.copy_predicated(out=cand[:], mask=eq[:], data=val_sb[:])

            top8 = small.tile([P, 8], I32, tag=f"top8_{d}")
            nc.vector.max(out=top8[:], in_=cand[:])

            top8f = small.tile([P, 8], F32, tag=f"top8f_{d}")
            nc.vector.tensor_copy(out=top8f[:], in_=top8[:])

            comp_in = small.tile([16, 64], F32, tag=f"compin_{d}")
            nc.sync.dma_start(comp_in[:], top8f[:])

            comp_out = small.tile([16, CAP // 16], F32, tag=f"compout_{d}")
            nfound = small.tile([1, 1], mybir.dt.uint32, tag=f"nf_{d}")
            nc.gpsimd.sparse_gather(
                out=comp_out[:], in_=comp_in[:], num_found=nfound[:]
            )

            # the instruction writes junk past num_found; rebuild the -1 tail
            comp_row = small.tile([1, CAP], F32, tag=f"comprow_{d}")
            nc.sync.dma_start(comp_row[:], comp_out[:, 0:1])
            nf_f = small.tile([1, 1], F32, tag=f"nff_{d}")
            nc.vector.tensor_copy(out=nf_f[:], in_=nfound[:])
            mask_row = small.tile([1, CAP], I32, tag=f"maskrow_{d}")
            nc.vector.tensor_scalar(
                out=mask_row[:], in0=iota_row[:], scalar1=nf_f[:, :1],
                scalar2=None, op0=AluOp.is_lt,
            )
            masked = small.tile([1, CAP], F32, tag=f"masked_{d}")
            nc.gpsimd.memset(masked[:], -1.0)
            nc.vector.copy_predicated(out=masked[:], mask=mask_row[:], data=comp_row[:])
            masked_tiles[d] = masked

            if not os.environ.get("KERNEL_P2P"):
                nc.sync.dma_start(cc_in[di * CAP:(di + 1) * CAP], masked[:])

            if "dbg_top8_f" in outs:
                nc.sync.dma_start(outs[f"dbg_top8_{d}"][:], top8f[:])
                nc.sync.dma_start(outs[f"dbg_comp_{d}"][:], masked[:])

        # ---- all-gather the candidate lists ------------------------------------
        ids_cols = None
        if os.environ.get("KERNEL_P2P"):
            # XOR-butterfly all-gather over SBUF remote DMA: 3 rounds with
            # partner tpb ^ step.  Column order of contributions is an
            # XOR-permutation of ranks, which is fine — the union of candidate
            # slots is order-insensitive downstream.
            rsem = nc.monotonic_semaphore(0)
            lsem = nc.monotonic_semaphore(1)
            cc_sb = big.tile([P, NCORES], F32, tag="cc_sb")
            nc.gpsimd.memset(cc_sb[:, 0:1], -1.0)
            nc.sync.dma_start(cc_sb[0:16, 0:1], masked_tiles["f"][:])
            nc.sync.dma_start(cc_sb[16:32, 0:1], masked_tiles["b"][:])
            ids_cols = {}
            for d2 in ("f", "b"):
                ids_cols[d2] = small.tile([P, 1], F32, tag=f"ids_{d2}",
                                          name=f"idscol_{d2}")
            with tc.tile_critical():
                for step in (1, 2, 4):
                    rdests = [None] * NCORES
                    rdests[step] = (0, step)
                    nc.gpsimd.remote_dma_broadcast(
                        out_ap=cc_sb[:, step:2 * step],
                        in_ap=cc_sb[:, 0:step],
                        remote_sem=rsem.sem(),
                        local_sem=lsem.sem(),
                        rdests=rdests,
                    )
                    nc.gpsimd.trigger_dma(count=1)
                    rsem.wait_inc(16 // NCORES)
                # relayout while still ordered after the final wait
                dsem = nc.monotonic_semaphore(2)
                nc.gpsimd.dma_start(
                    ids_cols["f"][:], cc_sb[0:16, :]).then_inc(dsem.sem(), 16)
                nc.gpsimd.dma_start(
                    ids_cols["b"][:], cc_sb[16:32, :]).then_inc(dsem.sem(), 16)
                dsem.wait_inc(32)
        elif os.environ.get("KERNEL_NO_CC"):
            # timing experiment only: skip the collective (wrong results)
            nc.sync.dma_start(cc_out[0:2 * CAP], cc_in[:])
        else:
            nc.gpsimd.collective_compute(
                "AllGather",
                AluOp.bypass,
                replica_groups=[list(range(NCORES))],
                ins=[cc_in.opt()],
                outs=[cc_out.opt()],
            )
        cc_view = cc_out[:].rearrange("(r s) -> r s", s=2 * CAP)

        if "dbg_cc" in outs:
            cc_sb = small.tile([1, NCORES * 2 * CAP], F32, tag="cc_sb")
            nc.sync.dma_start(cc_sb[:], cc_out[:])
            nc.sync.dma_start(outs["dbg_cc"][:], cc_sb[:])

        # ---- phase B: gather candidate feature rows, tiny GAT math -------------
        # misc gather (parallel rows + op row) is independent of the collective
        mi_col = small.tile([P, 1], I32, tag="mi")
        nc.sync.dma_start(mi_col[:], ins["misc_t"][:])
        gam = big.tile([P, IN], F32, tag="gam")
        nc.gpsimd.indirect_dma_start(
            out=gam[:], out_offset=None, in_=ins["feat"][:],
            in_offset=bass.IndirectOffsetOnAxis(ap=mi_col[:, :1], axis=0),
        )
        gamT_ps = pp.tile([P, P], F32, tag="t128", bufs=2)
        nc.tensor.transpose(out=gamT_ps[:], in_=gam[:], identity=ident[:])
        gamT = big.tile([P, P], F32, tag="gamT")
        nc.vector.tensor_copy(out=gamT[:], in_=gamT_ps[:])

        # para = sum of first 64 gathered rows -> out[256:384]
        para_ps = pp.tile([P, 1], F32, tag="ps_small", bufs=4)
        nc.tensor.matmul(
            out=para_ps[:], lhsT=gam[0:64, :], rhs=ones[0:64, :1],
            start=True, stop=True,
        )
        para = small.tile([P, 1], F32, tag="para")
        nc.vector.tensor_copy(out=para[:], in_=para_ps[:])
        nc.sync.dma_start(out[256:384], para[:, 0:1])

        # feat[op] -> out[128:256]
        nc.sync.dma_start(out[128:256], gam[64:65, :])

        for di, d in enumerate(("f", "b")):
            if ids_cols is not None:
                ids_col = ids_cols[d]
            else:
                ids_col = small.tile([P, 1], F32, tag=f"ids_{d}")
                nc.sync.dma_start(
                    ids_col[:], cc_view[:, di * CAP:(di + 1) * CAP]
                )

            valid = small.tile([P, 1], F32, tag=f"valid_{d}")
            nc.vector.tensor_scalar(
                out=valid[:], in0=ids_col[:], scalar1=-0.5, scalar2=None,
                op0=AluOp.is_gt,
            )
            idx_col = small.tile([P, 1], I32, tag=f"idx_{d}")
            nc.vector.tensor_scalar(
                out=idx_col[:], in0=ids_col[:], scalar1=0.0,
                scalar2=float(N - 1), op0=AluOp.max, op1=AluOp.min,
            )

            ga = big.tile([P, IN], F32, tag=f"ga_{d}")
            nc.gpsimd.indirect_dma_start(
                out=ga[:], out_offset=None, in_=ins["feat"][:],
                in_offset=bass.IndirectOffsetOnAxis(ap=idx_col[:, :1], axis=0),
            )
            gaT_ps = pp.tile([P, P], F32, tag="t128", bufs=2)
            nc.tensor.transpose(out=gaT_ps[:], in_=ga[:], identity=ident[:])
            gaT = big.tile([P, P], F32, tag=f"gaT_{d}")
            nc.vector.tensor_copy(out=gaT[:], in_=gaT_ps[:])

            # wl/wr = W @ [A_l_bd | A_r_bd]  -> [IN, 4]
            WT_sb = big.tile([P, P], F32, tag=f"WT_{d}")
            nc.sync.dma_start(WT_sb[:], ins[f"WT_{d}"][:])
            Acat = small.tile([P, 4], F32, tag=f"Acat_{d}")
            nc.sync.dma_start(Acat[:], ins[f"A_{d}"][:])
            wlr_ps = pp.tile([P, 4], F32, tag="ps_small", bufs=4)
            nc.tensor.matmul(
                out=wlr_ps[:], lhsT=WT_sb[:], rhs=Acat[:], start=True, stop=True
            )
            wlr = small.tile([P, 4], F32, tag=f"wlr_{d}")
            nc.vector.tensor_copy(out=wlr[:], in_=wlr_ps[:])

            # scores over candidates: rows 0:2 el, rows 2:4 er
            sc_ps = pp.tile([4, P], F32, tag="ps_small", bufs=4)
            nc.tensor.matmul(
                out=sc_ps[:], lhsT=wlr[:], rhs=gaT[:], start=True, stop=True
            )
            # er at op: from misc gather (op at slot 64); lhsT = wr columns only
            scm_ps = pp.tile([2, P], F32, tag="ps_small", bufs=4)
            nc.tensor.matmul(
                out=scm_ps[:], lhsT=wlr[:, 2:4], rhs=gamT[:], start=True, stop=True
            )
            er_col = small.tile([2, 1], F32, tag=f"er_{d}")
            nc.vector.tensor_copy(out=er_col[:], in_=scm_ps[0:2, 64:65])

            # ee = leaky_relu(el + er_op); softmax without max-shift (small values)
            ee = small.tile([2, P], F32, tag=f"ee_{d}")
            nc.vector.tensor_scalar(
                out=ee[:], in0=sc_ps[0:2, :], scalar1=er_col[:, :1],
                scalar2=None, op0=AluOp.add,
            )
            ee2 = small.tile([2, P], F32, tag=f"ee2_{d}")
            nc.vector.tensor_scalar(
                out=ee2[:], in0=ee[:], scalar1=NEG_SLOPE, scalar2=None,
                op0=AluOp.mult,
            )
            eel = small.tile([2, P], F32, tag=f"eel_{d}")
            nc.vector.tensor_tensor(out=eel[:], in0=ee[:], in1=ee2[:], op=AluOp.max)
            ex = small.tile([2, P], F32, tag=f"ex_{d}")
            nc.scalar.activation(out=ex[:], in_=eel[:], func=ActFn.Exp)

            # mask invalid slots in transposed layout, denominator via PE
            exT_ps = pp.tile([P, 2], F32, tag="ps_small", bufs=4)
            nc.tensor.transpose(out=exT_ps[:], in_=ex[:], identity=ident[0:2, 0:2])
            exm = small.tile([P, 2], F32, tag=f"exm_{d}")
            nc.vector.tensor_scalar(
                out=exm[:], in0=exT_ps[:], scalar1=valid[:, :1], scalar2=None,
                op0=AluOp.mult,
            )
            den_ps = pp.tile([2, 1], F32, tag="ps_small", bufs=4)
            nc.tensor.matmul(
                out=den_ps[:], lhsT=exm[:], rhs=ones[:, :1], start=True, stop=True
            )
            den = small.tile([2, 1], F32, tag=f"den_{d}")
            nc.vector.tensor_scalar(
                out=den[:], in0=den_ps[:], scalar1=1e-30, scalar2=None,
                op0=AluOp.add,
            )
            rden = small.tile([2, 1], F32, tag=f"rden_{d}")
            nc.vector.reciprocal(out=rden[:], in_=den[:])

            # unnormalized weighted feature sum, then normalize per head
            gu_ps = pp.tile([2, IN], F32, tag="ps_small", bufs=4)
            nc.tensor.matmul(
                out=gu_ps[:], lhsT=exm[:], rhs=ga[:], start=True, stop=True
            )
            gn = small.tile([2, IN], F32, tag=f"gn_{d}")
            nc.vector.tensor_scalar(
                out=gn[:], in0=gu_ps[:], scalar1=rden[:, :1], scalar2=None,
                op0=AluOp.mult,
            )
            gnT_ps = pp.tile([P, 2], F32, tag="ps_small", bufs=4)
            nc.tensor.transpose(out=gnT_ps[:], in_=gn[:], identity=ident[0:2, 0:2])
            gnT = small.tile([P, 2], F32, tag=f"gnT_{d}")
            nc.vector.tensor_copy(out=gnT[:], in_=gnT_ps[:])

            # head-sum fused into PSUM accumulation:
            # o_ps[0, :] = g_0 @ W[:, 0:64] + g_1 @ W[:, 64:128]
            W_sb = big.tile([P, P], F32, tag=f"W_{d}")
            nc.sync.dma_start(W_sb[:], ins[f"W_{d}"][:])
            o_ps = pp.tile([1, D], F32, tag="ps_small", bufs=4)
            nc.tensor.matmul(
                out=o_ps[:], lhsT=gnT[:, 0:1], rhs=W_sb[:, 0:D],
                start=True, stop=False,
            )
            nc.tensor.matmul(
                out=o_ps[:], lhsT=gnT[:, 1:2], rhs=W_sb[:, D:2 * D],
                start=False, stop=True,
            )

            bias_sb = small.tile([1, P], F32, tag=f"bias_{d}")
            nc.sync.dma_start(bias_sb[:], ins[f"bias_{d}"][:])
            bsum = small.tile([1, D], F32, tag=f"bsum_{d}")
            nc.vector.tensor_tensor(
                out=bsum[:], in0=bias_sb[0:1, 0:D], in1=bias_sb[0:1, D:2 * D],
                op=AluOp.add,
            )
            osum = small.tile([1, D], F32, tag=f"osum_{d}")
            nc.vector.tensor_tensor(out=osum[:], in0=o_ps[:], in1=bsum[:], op=AluOp.add)
            orow = small.tile([1, D], F32, tag=f"orow_{d}")
            nc.vector.tensor_scalar(
                out=orow[:], in0=osum[:], scalar1=0.5, scalar2=None, op0=AluOp.mult
            )
            nc.sync.dma_start(out[di * D:(di + 1) * D], orow[:])


_IN_SPECS = [
    ("src_t", (P, COLS), np.int32),
    ("dst_t", (P, COLS), np.int32),
    ("op_t", (P, 1), np.float32),
    ("misc_t", (P, 1), np.int32),
    ("feat", (N, IN), np.float32),
    ("W_f", (IN, H * D), np.float32),
    ("WT_f", (H * D, IN), np.float32),
    ("A_f", (H * D, 4), np.float32),
    ("bias_f", (1, H * D), np.float32),
    ("W_b", (IN, H * D), np.float32),
    ("WT_b", (H * D, IN), np.float32),
    ("A_b", (H * D, 4), np.float32),
    ("bias_b", (1, H * D), np.float32),
]


def build_nc(debug_outs=False):
    nc = bacc.Bacc(
        "TRN2",
        target_bir_lowering=False,
        debug=False,
        enable_asserts=True,
        num_devices=NCORES,
        monotonic_sem_count=3,
    )
    ins = {
        name: nc.dram_tensor(name, list(shape), mybir.dt.from_np(np.dtype(dt)),
                             kind="ExternalInput").ap()
        for name, shape, dt in _IN_SPECS
    }
    outs = {
        "out": nc.dram_tensor("out", [4 * IN - H * D], F32,
                              kind="ExternalOutput").ap()
    }
    if debug_outs:
        for d in ("f", "b"):
            outs[f"dbg_top8_{d}"] = nc.dram_tensor(
                f"dbg_top8_{d}", [P, 8], F32, kind="ExternalOutput").ap()
            outs[f"dbg_comp_{d}"] = nc.dram_tensor(
                f"dbg_comp_{d}", [16], F32, kind="ExternalOutput").ap()
        outs["dbg_cc"] = nc.dram_tensor(
            "dbg_cc", [NCORES * 2 * CAP], F32, kind="ExternalOutput").ap()
    with tile.TileContext(nc) as tc:
        build_body(nc, tc, outs, ins)
    nc.compile()
    return nc


def _block_diag_a(a_l, a_r):
    """[H,D] x2 -> [H*D, 4] with A[h*D+d, h] = a_l[h,d], A[h*D+d, 2+h] = a_r[h,d]."""
    A = np.zeros((H * D, 4), np.float32)
    for h in range(H):
        A[h * D:(h + 1) * D, h] = a_l[h]
        A[h * D:(h + 1) * D, 2 + h] = a_r[h]
    return A


def shard_inputs(feat, W_f, a_l_f, a_r_f, bias_f, W_b, a_l_b, a_r_b, bias_b,
                 src, dst, op, parallel):
    feat = np.ascontiguousarray(np.asarray(feat, np.float32))
    src = np.asarray(src, np.int32).ravel()
    dst = np.asarray(dst, np.int32).ravel()
    opv = int(np.asarray(op).item())
    parallel = np.asarray(parallel, np.int32).ravel()

    op_t = np.full((P, 1), opv, np.float32)
    misc = np.zeros((P, 1), np.int32)
    misc[:64, 0] = parallel
    misc[64, 0] = opv

    common = {
        "op_t": op_t,
        "misc_t": misc,
        "feat": feat,
        "W_f": np.ascontiguousarray(np.asarray(W_f, np.float32)),
        "WT_f": np.ascontiguousarray(np.asarray(W_f, np.float32).T),
        "A_f": _block_diag_a(np.asarray(a_l_f, np.float32),
                             np.asarray(a_r_f, np.float32)),
        "bias_f": np.asarray(bias_f, np.float32).reshape(1, H * D),
        "W_b": np.ascontiguousarray(np.asarray(W_b, np.float32)),
        "WT_b": np.ascontiguousarray(np.asarray(W_b, np.float32).T),
        "A_b": _block_diag_a(np.asarray(a_l_b, np.float32),
                             np.asarray(a_r_b, np.float32)),
        "bias_b": np.asarray(bias_b, np.float32).reshape(1, H * D),
    }

    in_maps = []
    pad = np.full(PADC - EPC, -1, np.int32)
    for m in range(NCORES):
        sl = slice(m * EPC, (m + 1) * EPC)
        in_maps.append({
            "src_t": np.concatenate([src[sl], pad]).reshape(P, COLS),
            "dst_t": np.concatenate([dst[sl], pad]).reshape(P, COLS),
            **common,
        })
    return in_maps


_NC_CACHE = {}


def get_nc():
    if "nc" not in _NC_CACHE:
        _NC_CACHE["nc"] = build_nc()
    return _NC_CACHE["nc"]


def kernel(**inputs):
    nc = get_nc()
    in_maps = shard_inputs(**inputs)
    res = run_bass_kernel_spmd(
        nc, in_maps, core_ids=list(range(NCORES)),
        trace=bool(int(os.environ.get("KERNEL_TRACE", "0"))),
    )
    if int(os.environ.get("KERNEL_TRACE", "0")) and res.exec_time_ns is not None:
        print(f"HW exec time: {res.exec_time_ns} ns")
        _NC_CACHE["last_results"] = res
    return np.asarray(res.results[0]["out"])


# revision 26
# speedup vs baseline: 1.0901x; 1.0901x over previous
"""Trainium2 Bass kernel for nn_DevNet_63093069578584 (GAT row-op readout).

The reference computes two full GATConv layers (forward graph and reversed
graph) over N=100k nodes / E=1.6M edges but only reads row `op` of each
result, plus feat[op] and a 64-row feature sum.  Row `op` of a GAT depends
only on the edges incident to node `op` (expected ~16 of 1.6M), so the real
work is scanning the src/dst index arrays (2 x 6.4MB) for matches.

Distribution: edges are split evenly over 8 NeuronCores.  Each core scans
its chunk, extracts the matched neighbor ids (DVE top-8 per partition row,
then gpsimd sparse_gather compaction), the 8 cores AllGather their
candidate lists (32 floats each), and every core redundantly finishes the
tiny GAT math (indirect-DMA gather of <=128 feature rows + a few 128x128
matmuls) and writes the full [384] output.
"""

import os
import sys

import numpy as np

for _p in ("/opt/trn_rl_repo",):
    if _p not in sys.path:
        sys.path.insert(0, _p)

import concourse.bass as bass
import concourse.mybir as mybir
import concourse.tile as tile
from concourse import bacc
from concourse.bass_utils import run_bass_kernel_spmd
from concourse.masks import make_identity

# Problem constants (hardcoded per harness contract).
N = 100000
E = 1600000
IN = 128
H = 2
D = 64
NEG_SLOPE = 0.2
NCORES = 8
P = 128
EPC = E // NCORES          # edges per core = 200000
COLS = 1568                # free-dim columns; P*COLS = 200704 >= EPC
PADC = P * COLS
CAP = 16                   # candidate slots contributed per core per direction

F32 = mybir.dt.float32
I32 = mybir.dt.int32

AluOp = mybir.AluOpType
ActFn = mybir.ActivationFunctionType


def build_body(nc, tc, outs, ins):
    """Emit the kernel into TileContext `tc`.  `outs`/`ins` are dicts of DRAM APs."""
    out = outs["out"]

    with (
        tc.tile_pool(name="big", bufs=1) as big,
        tc.tile_pool(name="small", bufs=1) as small,
        tc.tile_pool(name="pp", bufs=1, space="PSUM") as pp,
        tc.tile_pool(name="dram", bufs=1, space="DRAM") as dram,
    ):
        # ---- shared small tiles -------------------------------------------------
        op_sb = small.tile([P, 1], F32, tag="op")
        nc.sync.dma_start(op_sb[:], ins["op_t"][:])

        ident = big.tile([P, P], F32, tag="ident")
        make_identity(nc, ident[:])

        ones = small.tile([P, 1], F32, tag="ones")
        nc.gpsimd.memset(ones[:], 1.0)

        # ---- phase A: scan edge chunks, per direction ---------------------------
        src_sb = big.tile([P, COLS], I32, tag="src")
        dst_sb = big.tile([P, COLS], I32, tag="dst")
        nc.sync.dma_start(src_sb[:], ins["src_t"][:])
        nc.sync.dma_start(dst_sb[:], ins["dst_t"][:])

        cc_in = dram.tile([2 * CAP], F32, tag="cc_in")
        cc_out = dram.tile([NCORES * 2 * CAP], F32, tag="cc_out")

        # slot index row for masking sparse_gather's garbage tail
        iota_row = small.tile([1, CAP], I32, tag="iota_row")
        nc.gpsimd.iota(iota_row[:], pattern=[[1, CAP]], base=0, channel_multiplier=0)

        masked_tiles = {}
        for di, d in enumerate(("f", "b")):
            cmp_sb = dst_sb if d == "f" else src_sb
            val_sb = src_sb if d == "f" else dst_sb

            eq = big.tile([P, COLS], I32, tag=f"eq_{d}")
            nc.vector.tensor_scalar(
                out=eq[:], in0=cmp_sb[:], scalar1=op_sb[:, :1], scalar2=None,
                op0=AluOp.is_equal,
            )

            cand = big.tile([P, COLS], I32, tag=f"cand_{d}")
            nc.gpsimd.memset(cand[:], -1)
            nc.vector.copy_predicated(out=cand[:], mask=eq[:], data=val_sb[:])

            top8 = small.tile([P, 8], I32, tag=f"top8_{d}")
            nc.vector.max(out=top8[:], in_=cand[:])

            top8f = small.tile([P, 8], F32, tag=f"top8f_{d}")
            nc.vector.tensor_copy(out=top8f[:], in_=top8[:])

            comp_in = small.tile([16, 64], F32, tag=f"compin_{d}")
            nc.sync.dma_start(comp_in[:], top8f[:])

            comp_out = small.tile([16, CAP // 16], F32, tag=f"compout_{d}")
            nfound = small.tile([1, 1], mybir.dt.uint32, tag=f"nf_{d}")
            nc.gpsimd.sparse_gather(
                out=comp_out[:], in_=comp_in[:], num_found=nfound[:]
            )

            # the instruction writes junk past num_found; rebuild the -1 tail
            comp_row = small.tile([1, CAP], F32, tag=f"comprow_{d}")
            nc.sync.dma_start(comp_row[:], comp_out[:, 0:1])
            nf_f = small.tile([1, 1], F32, tag=f"nff_{d}")
            nc.vector.tensor_copy(out=nf_f[:], in_=nfound[:])
            mask_row = small.tile([1, CAP], I32, tag=f"maskrow_{d}")
            nc.vector.tensor_scalar(
                out=mask_row[:], in0=iota_row[:], scalar1=nf_f[:, :1],
                scalar2=None, op0=AluOp.is_lt,
            )
            masked = small.tile([1, CAP], F32, tag=f"masked_{d}")
            nc.gpsimd.memset(masked[:], -1.0)
            nc.vector.copy_predicated(out=masked[:], mask=mask_row[:], data=comp_row[:])
            masked_tiles[d] = masked

            if not os.environ.get("KERNEL_P2P"):
                nc.sync.dma_start(cc_in[di * CAP:(di + 1) * CAP], masked[:])

            if "dbg_top8_f" in outs:
                nc.sync.dma_start(outs[f"dbg_top8_{d}"][:], top8f[:])
                nc.sync.dma_start(outs[f"dbg_comp_{d}"][:], masked[:])

        # ---- all-gather the candidate lists ------------------------------------
        ids_cols = None
        if os.environ.get("KERNEL_P2P"):
            # XOR-butterfly all-gather over SBUF remote DMA: 3 rounds with
            # partner tpb ^ step.  Column order of contributions is an
            # XOR-permutation of ranks, which is fine — the union of candidate
            # slots is order-insensitive downstream.
            rsem = nc.monotonic_semaphore(0)
            lsem = nc.monotonic_semaphore(1)
            cc_sb = big.tile([P, NCORES], F32, tag="cc_sb")
            nc.gpsimd.memset(cc_sb[:, 0:1], -1.0)
            nc.sync.dma_start(cc_sb[0:16, 0:1], masked_tiles["f"][:])
            nc.sync.dma_start(cc_sb[16:32, 0:1], masked_tiles["b"][:])
            ids_cols = {}
            for d2 in ("f", "b"):
                ids_cols[d2] = small.tile([P, 1], F32, tag=f"ids_{d2}",
                                          name=f"idscol_{d2}")
            with tc.tile_critical():
                for step in (1, 2, 4):
                    rdests = [None] * NCORES
                    rdests[step] = (0, step)
                    nc.gpsimd.remote_dma_broadcast(
                        out_ap=cc_sb[:, step:2 * step],
                        in_ap=cc_sb[:, 0:step],
                        remote_sem=rsem.sem(),
                        local_sem=lsem.sem(),
                        rdests=rdests,
                    )
                    nc.gpsimd.trigger_dma(count=1)
                    rsem.wait_inc(16 // NCORES)
                # relayout while still ordered after the final wait
                dsem = nc.monotonic_semaphore(2)
                nc.gpsimd.dma_start(
                    ids_cols["f"][:], cc_sb[0:16, :]).then_inc(dsem.sem(), 16)
                nc.gpsimd.dma_start(
                    ids_cols["b"][:], cc_sb[16:32, :]).then_inc(dsem.sem(), 16)
                dsem.wait_inc(32)
        elif os.environ.get("KERNEL_NO_CC"):
            # timing experiment only: skip the collective (wrong results)
            nc.sync.dma_start(cc_out[0:2 * CAP], cc_in[:])
        else:
            nc.gpsimd.collective_compute(
                "AllGather",
                AluOp.bypass,
                replica_groups=[list(range(NCORES))],
                ins=[cc_in.opt()],
                outs=[cc_out.opt()],
            )
        cc_view = cc_out[:].rearrange("(r s) -> r s", s=2 * CAP)

        if "dbg_cc" in outs:
            cc_sb = small.tile([1, NCORES * 2 * CAP], F32, tag="cc_sb")
            nc.sync.dma_start(cc_sb[:], cc_out[:])
            nc.sync.dma_start(outs["dbg_cc"][:], cc_sb[:])

        # ---- phase B: gather candidate feature rows, tiny GAT math -------------
        # misc gather (parallel rows + op row) is independent of the collective
        mi_col = small.tile([P, 1], I32, tag="mi")
        nc.sync.dma_start(mi_col[:], ins["misc_t"][:])
        gam = big.tile([P, IN], F32, tag="gam")
        nc.gpsimd.memset(gam[:], 0.0)
        nc.gpsimd.indirect_dma_start(
            out=gam[:], out_offset=None, in_=ins["feat"][:],
            in_offset=bass.IndirectOffsetOnAxis(ap=mi_col[:, :1], axis=0),
            bounds_check=N - 1, oob_is_err=False,
        )
        gamT_ps = pp.tile([P, P], F32, tag="t128", bufs=2)
        nc.tensor.transpose(out=gamT_ps[:], in_=gam[:], identity=ident[:])
        gamT = big.tile([P, P], F32, tag="gamT")
        nc.vector.tensor_copy(out=gamT[:], in_=gamT_ps[:])

        # para = sum of first 64 gathered rows -> out[256:384]
        para_ps = pp.tile([P, 1], F32, tag="ps_small", bufs=4)
        nc.tensor.matmul(
            out=para_ps[:], lhsT=gam[0:64, :], rhs=ones[0:64, :1],
            start=True, stop=True,
        )
        para = small.tile([P, 1], F32, tag="para")
        nc.vector.tensor_copy(out=para[:], in_=para_ps[:])
        nc.sync.dma_start(out[256:384], para[:, 0:1])

        # feat[op] -> out[128:256]
        nc.sync.dma_start(out[128:256], gam[64:65, :])

        for di, d in enumerate(("f", "b")):
            if ids_cols is not None:
                ids_col = ids_cols[d]
            else:
                ids_col = small.tile([P, 1], F32, tag=f"ids_{d}")
                nc.sync.dma_start(
                    ids_col[:], cc_view[:, di * CAP:(di + 1) * CAP]
                )

            valid = small.tile([P, 1], F32, tag=f"valid_{d}")
            nc.vector.tensor_scalar(
                out=valid[:], in0=ids_col[:], scalar1=-0.5, scalar2=None,
                op0=AluOp.is_gt,
            )
            # invalid slots (-1) -> index 1e6: skipped by the bounds check
            skipoff = small.tile([P, 1], F32, tag=f"skipoff_{d}")
            nc.vector.tensor_scalar(
                out=skipoff[:], in0=valid[:], scalar1=-1.0e6, scalar2=1.0e6,
                op0=AluOp.mult, op1=AluOp.add,
            )
            idx_col = small.tile([P, 1], I32, tag=f"idx_{d}")
            nc.vector.tensor_tensor(
                out=idx_col[:], in0=ids_col[:], in1=skipoff[:], op=AluOp.add,
            )

            ga = big.tile([P, IN], F32, tag=f"ga_{d}")
            nc.gpsimd.memset(ga[:], 0.0)
            nc.gpsimd.indirect_dma_start(
                out=ga[:], out_offset=None, in_=ins["feat"][:],
                in_offset=bass.IndirectOffsetOnAxis(ap=idx_col[:, :1], axis=0),
                bounds_check=N - 1, oob_is_err=False,
            )
            gaT_ps = pp.tile([P, P], F32, tag="t128", bufs=2)
            nc.tensor.transpose(out=gaT_ps[:], in_=ga[:], identity=ident[:])
            gaT = big.tile([P, P], F32, tag=f"gaT_{d}")
            nc.vector.tensor_copy(out=gaT[:], in_=gaT_ps[:])

            # wl/wr = W @ [A_l_bd | A_r_bd]  -> [IN, 4]
            WT_sb = big.tile([P, P], F32, tag=f"WT_{d}")
            nc.sync.dma_start(WT_sb[:], ins[f"WT_{d}"][:])
            Acat = small.tile([P, 4], F32, tag=f"Acat_{d}")
            nc.sync.dma_start(Acat[:], ins[f"A_{d}"][:])
            wlr_ps = pp.tile([P, 4], F32, tag="ps_small", bufs=4)
            nc.tensor.matmul(
                out=wlr_ps[:], lhsT=WT_sb[:], rhs=Acat[:], start=True, stop=True
            )
            wlr = small.tile([P, 4], F32, tag=f"wlr_{d}")
            nc.vector.tensor_copy(out=wlr[:], in_=wlr_ps[:])

            # scores over candidates: rows 0:2 el, rows 2:4 er
            sc_ps = pp.tile([4, P], F32, tag="ps_small", bufs=4)
            nc.tensor.matmul(
                out=sc_ps[:], lhsT=wlr[:], rhs=gaT[:], start=True, stop=True
            )
            # er at op: from misc gather (op at slot 64); lhsT = wr columns only
            scm_ps = pp.tile([2, P], F32, tag="ps_small", bufs=4)
            nc.tensor.matmul(
                out=scm_ps[:], lhsT=wlr[:, 2:4], rhs=gamT[:], start=True, stop=True
            )
            er_col = small.tile([2, 1], F32, tag=f"er_{d}")
            nc.vector.tensor_copy(out=er_col[:], in_=scm_ps[0:2, 64:65])

            # ee = leaky_relu(el + er_op); softmax without max-shift (small values)
            ee = small.tile([2, P], F32, tag=f"ee_{d}")
            nc.vector.tensor_scalar(
                out=ee[:], in0=sc_ps[0:2, :], scalar1=er_col[:, :1],
                scalar2=None, op0=AluOp.add,
            )
            ee2 = small.tile([2, P], F32, tag=f"ee2_{d}")
            nc.vector.tensor_scalar(
                out=ee2[:], in0=ee[:], scalar1=NEG_SLOPE, scalar2=None,
                op0=AluOp.mult,
            )
            eel = small.tile([2, P], F32, tag=f"eel_{d}")
            nc.vector.tensor_tensor(out=eel[:], in0=ee[:], in1=ee2[:], op=AluOp.max)
            ex = small.tile([2, P], F32, tag=f"ex_{d}")
            nc.scalar.activation(out=ex[:], in_=eel[:], func=ActFn.Exp)

            # mask invalid slots in transposed layout, denominator via PE
            exT_ps = pp.tile([P, 2], F32, tag="ps_small", bufs=4)
            nc.tensor.transpose(out=exT_ps[:], in_=ex[:], identity=ident[0:2, 0:2])
            exm = small.tile([P, 2], F32, tag=f"exm_{d}")
            nc.vector.tensor_scalar(
                out=exm[:], in0=exT_ps[:], scalar1=valid[:, :1], scalar2=None,
                op0=AluOp.mult,
            )
            den_ps = pp.tile([2, 1], F32, tag="ps_small", bufs=4)
            nc.tensor.matmul(
                out=den_ps[:], lhsT=exm[:], rhs=ones[:, :1], start=True, stop=True
            )
            den = small.tile([2, 1], F32, tag=f"den_{d}")
            nc.vector.tensor_scalar(
                out=den[:], in0=den_ps[:], scalar1=1e-30, scalar2=None,
                op0=AluOp.add,
            )
            rden = small.tile([2, 1], F32, tag=f"rden_{d}")
            nc.vector.reciprocal(out=rden[:], in_=den[:])

            # unnormalized weighted feature sum, then normalize per head
            gu_ps = pp.tile([2, IN], F32, tag="ps_small", bufs=4)
            nc.tensor.matmul(
                out=gu_ps[:], lhsT=exm[:], rhs=ga[:], start=True, stop=True
            )
            gn = small.tile([2, IN], F32, tag=f"gn_{d}")
            nc.vector.tensor_scalar(
                out=gn[:], in0=gu_ps[:], scalar1=rden[:, :1], scalar2=None,
                op0=AluOp.mult,
            )
            gnT_ps = pp.tile([P, 2], F32, tag="ps_small", bufs=4)
            nc.tensor.transpose(out=gnT_ps[:], in_=gn[:], identity=ident[0:2, 0:2])
            gnT = small.tile([P, 2], F32, tag=f"gnT_{d}")
            nc.vector.tensor_copy(out=gnT[:], in_=gnT_ps[:])

            # head-sum fused into PSUM accumulation:
            # o_ps[0, :] = g_0 @ W[:, 0:64] + g_1 @ W[:, 64:128]
            W_sb = big.tile([P, P], F32, tag=f"W_{d}")
            nc.sync.dma_start(W_sb[:], ins[f"W_{d}"][:])
            o_ps = pp.tile([1, D], F32, tag="ps_small", bufs=4)
            nc.tensor.matmul(
                out=o_ps[:], lhsT=gnT[:, 0:1], rhs=W_sb[:, 0:D],
                start=True, stop=False,
            )
            nc.tensor.matmul(
                out=o_ps[:], lhsT=gnT[:, 1:2], rhs=W_sb[:, D:2 * D],
                start=False, stop=True,
            )

            bias_sb = small.tile([1, P], F32, tag=f"bias_{d}")
            nc.sync.dma_start(bias_sb[:], ins[f"bias_{d}"][:])
            bsum = small.tile([1, D], F32, tag=f"bsum_{d}")
            nc.vector.tensor_tensor(
                out=bsum[:], in0=bias_sb[0:1, 0:D], in1=bias_sb[0:1, D:2 * D],
                op=AluOp.add,
            )
            osum = small.tile([1, D], F32, tag=f"osum_{d}")
            nc.vector.tensor_tensor(out=osum[:], in0=o_ps[:], in1=bsum[:], op=AluOp.add)
            orow = small.tile([1, D], F32, tag=f"orow_{d}")
            nc.vector.tensor_scalar(
                out=orow[:], in0=osum[:], scalar1=0.5, scalar2=None, op0=AluOp.mult
            )
            nc.sync.dma_start(out[di * D:(di + 1) * D], orow[:])


_IN_SPECS = [
    ("src_t", (P, COLS), np.int32),
    ("dst_t", (P, COLS), np.int32),
    ("op_t", (P, 1), np.float32),
    ("misc_t", (P, 1), np.int32),
    ("feat", (N, IN), np.float32),
    ("W_f", (IN, H * D), np.float32),
    ("WT_f", (H * D, IN), np.float32),
    ("A_f", (H * D, 4), np.float32),
    ("bias_f", (1, H * D), np.float32),
    ("W_b", (IN, H * D), np.float32),
    ("WT_b", (H * D, IN), np.float32),
    ("A_b", (H * D, 4), np.float32),
    ("bias_b", (1, H * D), np.float32),
]


def build_nc(debug_outs=False):
    nc = bacc.Bacc(
        "TRN2",
        target_bir_lowering=False,
        debug=False,
        enable_asserts=True,
        num_devices=NCORES,
        monotonic_sem_count=3,
    )
    ins = {
        name: nc.dram_tensor(name, list(shape), mybir.dt.from_np(np.dtype(dt)),
                             kind="ExternalInput").ap()
        for name, shape, dt in _IN_SPECS
    }
    outs = {
        "out": nc.dram_tensor("out", [4 * IN - H * D], F32,
                              kind="ExternalOutput").ap()
    }
    if debug_outs:
        for d in ("f", "b"):
            outs[f"dbg_top8_{d}"] = nc.dram_tensor(
                f"dbg_top8_{d}", [P, 8], F32, kind="ExternalOutput").ap()
            outs[f"dbg_comp_{d}"] = nc.dram_tensor(
                f"dbg_comp_{d}", [16], F32, kind="ExternalOutput").ap()
        outs["dbg_cc"] = nc.dram_tensor(
            "dbg_cc", [NCORES * 2 * CAP], F32, kind="ExternalOutput").ap()
    with tile.TileContext(nc) as tc:
        build_body(nc, tc, outs, ins)
    nc.compile()
    return nc


def _block_diag_a(a_l, a_r):
    """[H,D] x2 -> [H*D, 4] with A[h*D+d, h] = a_l[h,d], A[h*D+d, 2+h] = a_r[h,d]."""
    A = np.zeros((H * D, 4), np.float32)
    for h in range(H):
        A[h * D:(h + 1) * D, h] = a_l[h]
        A[h * D:(h + 1) * D, 2 + h] = a_r[h]
    return A


def shard_inputs(feat, W_f, a_l_f, a_r_f, bias_f, W_b, a_l_b, a_r_b, bias_b,
                 src, dst, op, parallel):
    feat = np.ascontiguousarray(np.asarray(feat, np.float32))
    src = np.asarray(src, np.int32).ravel()
    dst = np.asarray(dst, np.int32).ravel()
    opv = int(np.asarray(op).item())
    parallel = np.asarray(parallel, np.int32).ravel()

    op_t = np.full((P, 1), opv, np.float32)
    misc = np.full((P, 1), N, np.int32)
    misc[:64, 0] = parallel
    misc[64, 0] = opv

    common = {
        "op_t": op_t,
        "misc_t": misc,
        "feat": feat,
        "W_f": np.ascontiguousarray(np.asarray(W_f, np.float32)),
        "WT_f": np.ascontiguousarray(np.asarray(W_f, np.float32).T),
        "A_f": _block_diag_a(np.asarray(a_l_f, np.float32),
                             np.asarray(a_r_f, np.float32)),
        "bias_f": np.asarray(bias_f, np.float32).reshape(1, H * D),
        "W_b": np.ascontiguousarray(np.asarray(W_b, np.float32)),
        "WT_b": np.ascontiguousarray(np.asarray(W_b, np.float32).T),
        "A_b": _block_diag_a(np.asarray(a_l_b, np.float32),
                             np.asarray(a_r_b, np.float32)),
        "bias_b": np.asarray(bias_b, np.float32).reshape(1, H * D),
    }

    in_maps = []
    pad = np.full(PADC - EPC, -1, np.int32)
    for m in range(NCORES):
        sl = slice(m * EPC, (m + 1) * EPC)
        in_maps.append({
            "src_t": np.concatenate([src[sl], pad]).reshape(P, COLS),
            "dst_t": np.concatenate([dst[sl], pad]).reshape(P, COLS),
            **common,
        })
    return in_maps


_NC_CACHE = {}


def get_nc():
    if "nc" not in _NC_CACHE:
        _NC_CACHE["nc"] = build_nc()
    return _NC_CACHE["nc"]


def kernel(**inputs):
    nc = get_nc()
    in_maps = shard_inputs(**inputs)
    res = run_bass_kernel_spmd(
        nc, in_maps, core_ids=list(range(NCORES)),
        trace=bool(int(os.environ.get("KERNEL_TRACE", "0"))),
    )
    if int(os.environ.get("KERNEL_TRACE", "0")) and res.exec_time_ns is not None:
        print(f"HW exec time: {res.exec_time_ns} ns")
        _NC_CACHE["last_results"] = res
    return np.asarray(res.results[0]["out"])
